# revision 35
# baseline (speedup 1.0000x reference)
"""Trainium2 Bass kernel for NeuralFractionalDE.

out = x_current + drift(x)*DT + softplus_head(x)*(noise*DT^H) + frac_deriv*(ALPHA*DT)

where frac_deriv = sum_k (x_hist[:,k+1,:]-x_hist[:,k,:]) * w[k] collapses to
sum_t c[t] * x_hist[:,t,:] with c[t] = w[t-1]-w[t] (boundary adjusted).

Data parallel over 8 NeuronCores (256 batch rows each). The 1 GiB
x_history stream is contracted on the TensorEngine: time is laid out as
t = 8*p + ti (p = partition), so each partition streams contiguous 4 KiB
rows from HBM, and 8 accumulating [128,1]^T x [128,512] matmuls per psum
row perform the weighted time reduction.

The GpSimd (SWDGE) FIFO carries ONLY the 32 stream loads: everything
else lives on the scalar/sync HWDGE rings so descriptor generation for
the big stream is never blocked. The MLPs run on ACT with native
Tanh/Softplus tables (no DVE in the MLP path: DVE perf-mode ops lock
the shared DVE/GpSimd SBUF port and stall Q7 descriptor emission), and
their stages are interleaved into the stream loop so they never occupy
the head of the Tensor FIFO.
"""

import math

import numpy as np

try:
    import concourse.bass as bass
except ImportError:  # pragma: no cover
    import sys

    sys.path.insert(0, "/opt/trn_rl_repo")
    import concourse.bass as bass

import concourse.bacc as bacc
import concourse.mybir as mybir
import concourse.tile as tile
from concourse.bass_utils import run_bass_kernel_spmd

ALPHA = 0.7
K = 1024
DT = 0.01
H = 0.5 + ALPHA / 2
D = 128
HID = 256
B = 2048
N_CORES = 8
B_PER = B // N_CORES  # 256
TI = 8  # time sub-steps per partition: t = TI*p + ti
NB = 8  # batch rows per streamed x_history tile
G = B_PER // NB  # 32 groups; batch b = NB*g + bi
NCB = (NB * D) // 512  # matmul column chunks per (group, ti)

F32 = mybir.dt.float32
BF16 = mybir.dt.bfloat16
FP8 = mybir.dt.float8e4
AF = mybir.ActivationFunctionType
OP = mybir.AluOpType
PM = mybir.MatmulPerfMode

# fp8e4m3 coefficient pre-scale: c spans ~6 decades, so scale c_max (1.44e-3)
# toward fp8's normal range and undo with 1/CS in the psum->stage copy.
CS = float(2**17)
CM = 32  # DoubleRow LDWEIGHTS rejects M=1; pad the stationary to 32 columns


def _coeffs() -> np.ndarray:
    t = np.arange(1, K + 1, dtype=np.float32)
    kern = (t ** np.float32(-ALPHA)) / np.float32(math.gamma(1.0 - ALPHA))
    w = kern[::-1][: K - 1]  # w[k] = kern[K-1-k]
    c = np.zeros(K, dtype=np.float32)
    c[1:] += w
    c[: K - 1] -= w
    c *= np.float32(ALPHA * DT * CS)
    # t = 8p + 2*ti2 + j (j = DoubleRow k-pair = ti parity): the SBUF tile
    # keeps the contiguous (bi, ti, d) source order (4 KiB descriptors,
    # ~420 GB/s sustained vs 414 for the 2 KiB j-split) and the matmul AP
    # transposes (bi, j) to put the k-pair at dim 1.
    # cpad[p, ti2, j, 0] = c[8p + 2*ti2 + j].
    cpad = np.zeros((128, TI // 2, 2, CM), dtype=np.float32)
    cpad[:, :, :, 0] = c.reshape(128, TI // 2, 2)
    return cpad


def _build_program() -> bass.Bass:
    # Bacc (not raw Bass): its compile() legalizes semaphore waits to the
    # 1-wait-per-instruction ISA limit (generate_event_semaphores).
    nc = bacc.Bacc(None, target_bir_lowering=False)

    xh = nc.dram_tensor("xh", [B_PER, K, D], F32, kind="ExternalInput")
    xc = nc.dram_tensor("xc", [B_PER, D], F32, kind="ExternalInput")
    nz = nc.dram_tensor("nz", [B_PER], F32, kind="ExternalInput")
    wshapes = {
        "w1": [D, HID],
        "b1": [HID],
        "w2": [HID, HID],
        "b2": [HID],
        "w3": [HID, D],
        "b3": [D],
    }
    wd = {}
    for net in ("d", "g"):
        for nm, shp in wshapes.items():
            wd[net + nm] = nc.dram_tensor(net + nm, shp, F32, kind="ExternalInput")
    out = nc.dram_tensor("out", [B_PER, D], F32, kind="ExternalOutput")

    import ml_dtypes

    c8d = nc.inline_tensor(
        _coeffs().astype(ml_dtypes.float8_e4m3fn), name="c8const"
    )
    identd = nc.inline_tensor(np.eye(128, dtype=np.float32), name="identconst")

    with tile.TileContext(nc) as tc:
        with (
            tc.tile_pool(name="const", bufs=1) as cpool,
            tc.tile_pool(name="stream", bufs=8) as spool,
            tc.tile_pool(name="work", bufs=4) as wpool,
            tc.tile_pool(name="psf", bufs=4, space=bass.MemorySpace.PSUM) as psf,
            tc.tile_pool(name="psm", bufs=2, space=bass.MemorySpace.PSUM) as psm,
            tc.tile_pool(name="pst", bufs=2, space=bass.MemorySpace.PSUM) as pst,
        ):
            # ---- small constant loads, all on the scalar HWDGE ring ----
            c8_sb = cpool.tile([128, TI // 2, 2, CM], FP8, tag="c8")
            nc.scalar.dma_start(out=c8_sb[:], in_=c8d[:])
            ident_sb = cpool.tile([128, 128], F32, tag="ident")
            nc.scalar.dma_start(out=ident_sb[:], in_=identd[:])

            xc_sb = []
            nzf_sb = []
            for tb in range(2):
                t_ = cpool.tile([128, D], F32, tag=f"xc{tb}")
                nc.scalar.dma_start(out=t_[:], in_=xc[tb * 128 : (tb + 1) * 128, :])
                xc_sb.append(t_)
                n_ = cpool.tile([128, 1], F32, tag=f"nz{tb}")
                nc.scalar.dma_start(
                    out=n_[:],
                    in_=nz[tb * 128 : (tb + 1) * 128].rearrange("(p o) -> p o", o=1),
                )
                nzf_sb.append(n_)

            wsb = {}
            for net in ("d", "g"):
                w1 = cpool.tile([128, HID], F32, tag=f"{net}w1")
                nc.scalar.dma_start(out=w1[:], in_=wd[net + "w1"][:])
                w2 = []
                w3 = []
                b1 = []
                b2 = []
                for i in range(2):
                    t_ = cpool.tile([128, HID], F32, tag=f"{net}w2{i}")
                    nc.scalar.dma_start(
                        out=t_[:], in_=wd[net + "w2"][i * 128 : (i + 1) * 128, :]
                    )
                    w2.append(t_)
                    t_ = cpool.tile([128, D], F32, tag=f"{net}w3{i}")
                    nc.scalar.dma_start(
                        out=t_[:], in_=wd[net + "w3"][i * 128 : (i + 1) * 128, :]
                    )
                    w3.append(t_)
                    t_ = cpool.tile([128, 1], F32, tag=f"{net}b1{i}")
                    nc.scalar.dma_start(
                        out=t_[:],
                        in_=wd[net + "b1"][i * 128 : (i + 1) * 128].rearrange(
                            "(p o) -> p o", o=1
                        ),
                    )
                    b1.append(t_)
                    t_ = cpool.tile([128, 1], F32, tag=f"{net}b2{i}")
                    nc.scalar.dma_start(
                        out=t_[:],
                        in_=wd[net + "b2"][i * 128 : (i + 1) * 128].rearrange(
                            "(p o) -> p o", o=1
                        ),
                    )
                    b2.append(t_)
                b3 = cpool.tile([128, 1], F32, tag=f"{net}b3")
                nc.scalar.dma_start(
                    out=b3[:], in_=wd[net + "b3"][:].rearrange("(p o) -> p o", o=1)
                )
                wsb[net] = (w1, b1, w2, b2, w3, b3)

            # pre-scale the tail scalars on ACT: nzf = nz * DT^H,
            # db3 -> db3 * DT (drift head bias folds the *DT step scale)
            for tb in range(2):
                nc.scalar.activation(
                    nzf_sb[tb][:], nzf_sb[tb][:], AF.Copy, scale=float(DT**H)
                )
            nc.scalar.activation(
                wsb["d"][5][:], wsb["d"][5][:], AF.Copy, scale=float(DT)
            )

            # ---- x_current transpose: [b, d] -> [d, b] ----
            xcT_sb = cpool.tile([128, B_PER], F32, tag="xcT")
            for tb in range(2):
                pt = pst.tile([128, 128], F32, tag="pst")
                nc.tensor.transpose(pt[:], xc_sb[tb][:], ident_sb[:])
                nc.scalar.activation(
                    xcT_sb[:, tb * 128 : (tb + 1) * 128], pt[:], AF.Copy
                )

            # ---- the two MLPs, staged so each stage slots in between ----
            # stream groups. All nonlinearities are single native-ACT ops
            # (Tanh / Softplus LUT sets); the only DVE in the whole kernel
            # is the 2-op combine in each tail.
            driftT_sb = cpool.tile([128, B_PER], F32, tag="driftT")
            diffT_sb = cpool.tile([128, B_PER], F32, tag="diffT")
            mlp_state: dict = {}

            def l1_mm(net):
                w1 = wsb[net][0]
                ps_l = []
                for j in range(2):
                    ps = psm.tile([128, B_PER], F32, tag="psm")
                    nc.tensor.matmul(
                        ps[:],
                        w1[:, j * 128 : (j + 1) * 128],
                        xcT_sb[:],
                        start=True,
                        stop=True,
                    )
                    ps_l.append(ps)
                mlp_state[net + "ps1"] = ps_l

            def l1_act(net):
                b1 = wsb[net][1]
                h1 = []
                for j in range(2):
                    h = cpool.tile([128, B_PER], F32, tag=f"{net}h1{j}")
                    nc.scalar.activation(
                        h[:], mlp_state[net + "ps1"][j][:], AF.Tanh, bias=b1[j][:]
                    )
                    h1.append(h)
                mlp_state[net + "h1"] = h1

            def l2_mm(net):
                w2 = wsb[net][2]
                h1 = mlp_state[net + "h1"]
                ps_l = []
                for j in range(2):
                    ps = psm.tile([128, B_PER], F32, tag="psm")
                    for i in range(2):
                        nc.tensor.matmul(
                            ps[:],
                            w2[i][:, j * 128 : (j + 1) * 128],
                            h1[i][:],
                            start=(i == 0),
                            stop=(i == 1),
                        )
                    ps_l.append(ps)
                mlp_state[net + "ps2"] = ps_l

            def l2_act(net):
                b2 = wsb[net][3]
                h2 = []
                for j in range(2):
                    h = cpool.tile([128, B_PER], F32, tag=f"{net}h2{j}")
                    nc.scalar.activation(
                        h[:], mlp_state[net + "ps2"][j][:], AF.Tanh, bias=b2[j][:]
                    )
                    h2.append(h)
                mlp_state[net + "h2"] = h2

            def l3_head(net):
                w3, b3 = wsb[net][4], wsb[net][5]
                h2 = mlp_state[net + "h2"]
                ps = psm.tile([128, B_PER], F32, tag="psm")
                for i in range(2):
                    nc.tensor.matmul(
                        ps[:], w3[i][:], h2[i][:], start=(i == 0), stop=(i == 1)
                    )
                if net == "d":
                    # driftT = (raw + b3) * DT  (b3 pre-scaled by DT)
                    nc.scalar.activation(
                        driftT_sb[:], ps[:], AF.Identity, bias=b3[:], scale=float(DT)
                    )
                else:
                    # softplus = ln(1 + exp(x + b3)); no Softplus LUT exists,
                    # and the +1 folds into Ln's pre-bias (one table switch,
                    # exp_and_others -> natural_log_exp_and_others)
                    nc.scalar.activation(diffT_sb[:], ps[:], AF.Exp, bias=b3[:])
                    nc.scalar.activation(diffT_sb[:], diffT_sb[:], AF.Ln, bias=1.0)

            stages = [
                lambda: l1_mm("d"),
                lambda: l1_act("d"),
                lambda: l2_mm("d"),
                lambda: l2_act("d"),
                lambda: l3_head("d"),
                lambda: l1_mm("g"),
                lambda: l1_act("g"),
                lambda: l2_mm("g"),
                lambda: l2_act("g"),
                lambda: l3_head("g"),
            ]

            # per-half fractional accumulators, scattered into directly
            # (SBUF->SBUF on the scalar HWDGE ring; no DRAM roundtrip)
            fracb = [
                cpool.tile([128, D], F32, tag=f"fracb{tb}", name=f"fracb{tb}")
                for tb in range(2)
            ]

            # precompute o_partial = diffusion*(noise*DT^H) + drift*DT + xc
            # right after the MLP so the post-stream tail is just one DVE
            # add (fracb) + the output DMA
            opart = [
                cpool.tile([128, D], F32, tag=f"opart{tb}", name=f"opart{tb}")
                for tb in range(2)
            ]

            def precompute_out(tb):
                ptd = pst.tile([128, 128], F32, tag="pst")
                # ptd = drift_scaled^T + xc^T accumulated on PE
                nc.tensor.matmul(
                    ptd[:],
                    driftT_sb[:, tb * 128 : (tb + 1) * 128],
                    ident_sb[:],
                    start=True,
                    stop=False,
                )
                nc.tensor.matmul(
                    ptd[:],
                    xcT_sb[:, tb * 128 : (tb + 1) * 128],
                    ident_sb[:],
                    start=False,
                    stop=True,
                )
                ptg = pst.tile([128, 128], F32, tag="pst")
                nc.tensor.transpose(
                    ptg[:], diffT_sb[:, tb * 128 : (tb + 1) * 128], ident_sb[:]
                )
                # only one DVE input may read PSUM: stage ptd through SBUF
                nc.scalar.activation(opart[tb][:], ptd[:], AF.Copy)
                nc.vector.scalar_tensor_tensor(
                    out=opart[tb][:],
                    in0=ptg[:],
                    scalar=nzf_sb[tb][:],
                    in1=opart[tb][:],
                    op0=OP.mult,
                    op1=OP.add,
                )

            def do_tail(tb):
                o = wpool.tile([128, D], F32, tag="o")
                nc.vector.tensor_add(out=o[:], in0=opart[tb][:], in1=fracb[tb][:])
                nc.sync.dma_start(out=out[tb * 128 : (tb + 1) * 128, :], in_=o[:])

            # ---- fractional-derivative stream: the 128 MiB x_history scan ----
            # xh[b, TI*p + ti, d] -> tile[p, ti, bi, d] for b = NB*g + bi, so
            # each partition reads contiguous 4 KiB rows. The tile is cast
            # fp32 -> fp8e4 in-flight (SWDGE); the reduction runs DoubleRow
            # matmuls (0.5 cyc/row, k-pair = ti parity) accumulating fp32
            # PSUM, so the PE block stays well under the DMA time per group
            # even when HAM-throttled to 1.2 GHz and never paces the convoy.
            xh_r = xh.rearrange("(g bi) (p ti) d -> g p bi ti d", bi=NB, p=128)
            for g in range(G):
                xt = spool.tile([128, NB, TI, D], FP8, tag="xt")
                # Mid-stream: one 4 MiB DMA per group (4 KiB descriptors,
                # best sustained cast rate). Ramp groups and the final
                # group load in bi-halves: smaller first quanta fill the
                # ring faster, and the last group's first matmuls overlap
                # its in-flight second half (shorter tail).
                if 8 <= g < G - 1:
                    nc.gpsimd.dma_start(out=xt[:], in_=xh_r[g])
                else:
                    for h in range(2):
                        nc.gpsimd.dma_start(
                            out=xt[:, 4 * h : 4 * h + 4],
                            in_=xh_r[g, :, 4 * h : 4 * h + 4],
                        )
                stage = wpool.tile([1, NB * D], F32, tag="stage")
                for cb in range(NCB):
                    ps = psf.tile([CM, 512], F32, tag="psf")
                    for ti in range(TI // 2):
                        rhs = xt[
                            :, 4 * cb : 4 * cb + 4, 2 * ti : 2 * ti + 2, :
                        ].transpose([0, 2, 1, 3])
                        nc.tensor.matmul(
                            ps[:],
                            c8_sb[:, ti, :, :],
                            rhs,
                            start=(ti == 0),
                            stop=(ti == TI // 2 - 1),
                            perf_mode=PM.DoubleRow,
                        )
                    # row 0 holds the real column of the padded stationary;
                    # 1/CS undoes the fp8 coefficient pre-scale
                    nc.scalar.activation(
                        stage[0:1, cb * 512 : (cb + 1) * 512],
                        ps[0:1, :],
                        AF.Copy,
                        scale=float(1.0 / CS),
                    )
                # scatter rows b = NB*g + bi into the SBUF accumulator tile
                r0 = (g % 16) * NB
                nc.scalar.dma_start(
                    out=fracb[g // 16][r0 : r0 + NB, :],
                    in_=stage[0:1].rearrange("o (bi d) -> o bi d", bi=NB),
                )
                if g < len(stages):
                    stages[g]()
                elif g == len(stages):
                    precompute_out(0)
                elif g == len(stages) + 1:
                    precompute_out(1)
                if g == G // 2 - 1:
                    do_tail(0)
                elif g == G - 1:
                    do_tail(1)

    nc.compile()
    return nc


_NC_CACHE = None


def _get_program() -> bass.Bass:
    global _NC_CACHE
    if _NC_CACHE is None:
        _NC_CACHE = _build_program()
    return _NC_CACHE


def _in_maps(inputs: dict) -> list[dict]:
    f = lambda x: np.ascontiguousarray(np.asarray(x, dtype=np.float32))
    xh = f(inputs["x_history"])
    xc = f(inputs["x_current"])
    nz = f(inputs["noise"])
    assert xh.shape == (B, K, D) and xc.shape == (B, D) and nz.shape == (B,)
    rep = {}
    for net, pre in (("d", "d"), ("g", "g")):
        for nm in ("w1", "b1", "w2", "b2", "w3", "b3"):
            rep[net + nm] = f(inputs[pre + nm])
    maps = []
    for c in range(N_CORES):
        s = slice(c * B_PER, (c + 1) * B_PER)
        m = {"xh": xh[s], "xc": xc[s], "nz": nz[s]}
        m.update(rep)
        maps.append(m)
    return maps


def run(inputs: dict, trace: bool = False):
    nc = _get_program()
    res = run_bass_kernel_spmd(nc, _in_maps(inputs), list(range(N_CORES)), trace=trace)
    out = np.concatenate([res.results[c]["out"] for c in range(N_CORES)], axis=0)
    return out, res


def kernel(**inputs) -> np.ndarray:
    out, _ = run(inputs, trace=False)
    return out
